# revision 4
# baseline (speedup 1.0000x reference)
"""TRN2 Bass kernel for nn_Attention_15590731285136.

Computation (per batch b):
    g      = diag(W) * K[b]                       # [d]
    score  = relu(V[b] @ (g[:,None]*w1) + b1) @ w2 + b2   # [h]
    score  = where(mask[b], MASK_FILL, score)
    alpha  = softmax(score)                        # over h
    out[b] = alpha @ V[b]                          # [d]

Sharding: data-parallel over batch, 8 batches per core on 8 NeuronCores.

Key transformations (v3):
  * Token compaction: masked tokens have alpha == 0 exactly (their score
    is -2^32), so the host gathers only the unmasked tokens of each batch
    (~1024 of 2048) and pads to a multiple of 128. Padding tokens get the
    mask-fill score bias, so their alpha is exactly 0 too. This halves
    the fc1 GEMM, the relu/rowsum work, the alpha@V pass and the DMA
    traffic, with bit-identical math for the surviving tokens.
  * The elementwise gate and w2's magnitudes fold into the weight matrix
    host-side: w12[b] = g[b] * (w1[:, perm] * |w2[perm]|), with a
    sign-grouping permutation (positive-w2 columns first).
  * The fc1 GEMM runs in fp8 (e4m3) with MatmulPerfMode.DoubleRow: each
    matmul contracts TWO 128-deep k-slices per pass, 2x the fp16 rate.
    w12 is scaled by S (power of two) to sit in e4m3's dynamic range;
    softmax is invariant up to the final exp(score/S) which folds 1/S
    into the activation's scale operand.
  * All device-side layouts (compacted V^T fp8, compacted natural V fp16,
    gated w12, additive mask bias) are precomputed host-side, so every
    DMA is a plain contiguous row load.
  * relu+rowsum of fc1 runs fused on ScalarE (ACT, positive-w2 group)
    and VectorE (DVE, negative group) via accum_out; the first KSW token
    tiles' positive group also goes to DVE to balance the two engines.
  * The softmax denominator + alpha@V of batch i are emitted after the
    fc1 loop of batch i+1 (software pipelining) so the PE never waits
    for alpha.
"""

import numpy as np

B, H, D, HID = 64, 2048, 512, 512
NCORES = 8
BPC = B // NCORES          # batches per core
DC = D // 128              # 4 contraction chunks
MASK_FILL = -2.0**32 + 1.0
PRE = 5                    # batches of loads in flight ahead of compute
DEFER = 2                  # batches the softmax tail trails the fc1 loop
KSW = 2                    # leading token tiles whose pos-group runs on DVE


def _build(hp, b2val, inv_s, has_bias, htp):
    import concourse.mybir as mybir
    from concourse import bacc
    from concourse.tile import TileContext

    F32 = mybir.dt.float32
    F16 = mybir.dt.float16
    F8 = mybir.dt.float8e4
    ACTF = mybir.ActivationFunctionType
    ALU = mybir.AluOpType
    DR = mybir.MatmulPerfMode.DoubleRow

    HP = htp * 128             # padded token count
    DCH = DC * HP
    HTD = htp * D
    DCN = DC * HID

    nc = bacc.Bacc(trn_type="TRN2", num_devices=NCORES)

    # all inputs pre-arranged host-side into [128, cols] partition-major
    VT8 = nc.dram_tensor("VT8", (128, BPC * DCH), F8, kind="ExternalInput")
    V16 = nc.dram_tensor("V16", (128, BPC * HTD), F16, kind="ExternalInput")
    W12 = nc.dram_tensor("W12", (128, BPC * DCN), F8, kind="ExternalInput")
    MB = nc.dram_tensor("MB", (128, BPC * htp), F32, kind="ExternalInput")
    if has_bias:
        BI = nc.dram_tensor("BI", (1, HID), F16, kind="ExternalInput")
    OUT = nc.dram_tensor("OUT", (BPC, D), F32, kind="ExternalOutput")

    with TileContext(nc) as tc:
        with (
            tc.tile_pool(name="const", bufs=1) as cpool,
            tc.tile_pool(name="vt", bufs=PRE + 1) as vtpool,
            tc.tile_pool(name="v", bufs=PRE + DEFER + 1) as vpool,
            tc.tile_pool(name="w12", bufs=PRE + 1) as wpool,
            tc.tile_pool(name="scr", bufs=4) as scrpool,
            tc.tile_pool(name="small", bufs=8) as spool,
            tc.tile_pool(name="alpha", bufs=DEFER + 2) as alpool,
            tc.tile_pool(name="fin", bufs=4) as finpool,
            tc.tile_pool(name="fc1_ps", bufs=4, space="PSUM") as fc1ps,
            tc.tile_pool(name="tot_ps", bufs=2, space="PSUM") as totps,
            tc.tile_pool(name="acc_ps", bufs=2, space="PSUM") as accps,
        ):
            def emit_w(bi):
                w12 = wpool.tile([128, DCN], F8, tag="w12")
                nc.sync.dma_start(
                    out=w12, in_=W12.ap()[:, bi * DCN:(bi + 1) * DCN])
                vt = vtpool.tile([128, DCH], F8, tag="vt")
                nc.sync.dma_start(
                    out=vt, in_=VT8.ap()[:, bi * DCH:(bi + 1) * DCH])
                return vt, w12

            def emit_v(bi):
                v = vpool.tile([128, HTD], F16, tag="v")
                nc.gpsimd.dma_start(
                    out=v, in_=V16.ap()[:, bi * HTD:(bi + 1) * HTD])
                return v

            pend_w = [emit_w(bi) for bi in range(min(PRE, BPC))]
            pend_v = [emit_v(bi) for bi in range(min(PRE - 1, BPC))]

            # ---- one-time constants ----
            ones_col = cpool.tile([128, 1], F16, tag="ones")
            nc.vector.memset(ones_col, 1.0)
            mall = cpool.tile([128, BPC * htp], F32, tag="mall")
            nc.sync.dma_start(out=mall, in_=MB.ap())
            oball = cpool.tile([1, BPC * D], F32, tag="oball")
            if has_bias:
                ones_row = cpool.tile([1, 128], F16, tag="orr")
                nc.vector.memset(ones_row, 1.0)
                bias_sb = cpool.tile([1, HID], F16, tag="bias")
                nc.sync.dma_start(out=bias_sb, in_=BI.ap())

            def emit_tail(st):
                bi, alpha, v = st
                # alpha @ V
                acc = accps.tile([1, D], F32, tag="acc")
                for j in range(htp):
                    nc.tensor.matmul(
                        out=acc,
                        lhsT=alpha[:, j:j + 1],
                        rhs=v[:, j * D:(j + 1) * D],
                        start=(j == 0),
                        stop=(j == htp - 1),
                    )
                # denominator: sum over all tokens via PE + reduce
                tot = totps.tile([1, htp], F32, tag="tot")
                nc.tensor.matmul(out=tot, lhsT=ones_col, rhs=alpha,
                                 start=True, stop=True)
                tot_sb = finpool.tile([1, 1], F32, tag="tot_sb")
                nc.vector.tensor_reduce(
                    tot_sb, tot, axis=mybir.AxisListType.X, op=ALU.add)
                inv = finpool.tile([1, 1], F32, tag="inv")
                nc.vector.reciprocal(inv, tot_sb)
                nc.vector.tensor_scalar_mul(
                    oball[:, bi * D:(bi + 1) * D], acc, inv)

            deferred = []
            for bi in range(BPC):
                if bi + PRE < BPC:
                    pend_w.append(emit_w(bi + PRE))
                if bi + PRE - 1 < BPC:
                    pend_v.append(emit_v(bi + PRE - 1))
                vt, w12 = pend_w.pop(0)
                v = pend_v.pop(0)
                vt3 = vt.rearrange("p (c h) -> p c h", c=DC)
                w3 = w12.rearrange("p (c n) -> p c n", c=DC)
                mb = mall[:, bi * htp:(bi + 1) * htp]

                sp = spool.tile([128, htp], F32, tag="sp")
                sn = spool.tile([128, htp], F32, tag="sn")
                if hp == 0:
                    nc.vector.memset(sp, 0.0)
                if hp == HID:
                    nc.vector.memset(sn, 0.0)

                # ---- fc1 (fp8 DoubleRow) + fused relu/rowsum per tile ----
                for j in range(htp):
                    fc1 = fc1ps.tile([128, HID], F32, tag="fc1")
                    for pr in range(2):
                        nc.tensor.matmul(
                            out=fc1,
                            lhsT=vt3[:, 2 * pr:2 * pr + 2,
                                     j * 128:(j + 1) * 128],
                            rhs=w3[:, 2 * pr:2 * pr + 2, :],
                            start=(pr == 0),
                            stop=(pr == 1) and not has_bias,
                            perf_mode=DR,
                        )
                    if has_bias:
                        nc.tensor.matmul(
                            out=fc1, lhsT=ones_row, rhs=bias_sb,
                            start=False, stop=True,
                        )
                    if hp > 0:
                        if j < KSW:
                            scrp = scrpool.tile([128, HID], F16, tag="scrp")
                            nc.vector.tensor_scalar(
                                out=scrp[:, :hp], in0=fc1[:, :hp],
                                scalar1=0.0, scalar2=None,
                                op0=ALU.max, op1=ALU.add,
                                accum_out=sp[:, j:j + 1],
                            )
                        else:
                            scra = scrpool.tile([128, HID], F16, tag="scra")
                            nc.scalar.activation(
                                out=scra[:, :hp], in_=fc1[:, :hp],
                                func=ACTF.Relu,
                                accum_out=sp[:, j:j + 1],
                            )
                    if hp < HID:
                        scrd = scrpool.tile([128, HID], F16, tag="scrd")
                        nc.vector.tensor_scalar(
                            out=scrd[:, hp:], in0=fc1[:, hp:],
                            scalar1=0.0, scalar2=None,
                            op0=ALU.max, op1=ALU.add,
                            accum_out=sn[:, j:j + 1],
                        )

                # ---- scores -> masked -> exp(score/S) ----
                sc = spool.tile([128, htp], F32, tag="sc")
                nc.vector.tensor_sub(sc, sp, sn)
                scm = spool.tile([128, htp], F32, tag="scm")
                nc.vector.tensor_add(scm, sc, mb)
                alpha = alpool.tile([128, htp], F16, tag="alpha")
                nc.scalar.activation(
                    out=alpha, in_=scm, func=ACTF.Exp,
                    bias=float(b2val), scale=float(inv_s),
                )

                deferred.append((bi, alpha, v))
                if len(deferred) > DEFER:
                    emit_tail(deferred.pop(0))

            while deferred:
                emit_tail(deferred.pop(0))
            nc.sync.dma_start(
                out=OUT.ap().rearrange("b d -> (b d)")
                    .rearrange("(o f) -> o f", o=1),
                in_=oball)

    nc.finalize()
    return nc


def _prep(K, V, mask, W, w1, b1, w2, b2):
    """Host-side input marshalling (no device work)."""
    import ml_dtypes

    E4 = ml_dtypes.float8_e4m3   # TRN-style e4m3, max normal 240

    K = np.asarray(K, dtype=np.float32)
    V = np.asarray(V, dtype=np.float32)
    mask = np.asarray(mask).astype(bool)
    W = np.asarray(W, dtype=np.float32)
    w1 = np.asarray(w1, dtype=np.float32)
    b1 = np.asarray(b1, dtype=np.float32)
    w2 = np.asarray(w2, dtype=np.float32).reshape(-1)
    b2 = np.asarray(b2, dtype=np.float32).reshape(-1)

    g = np.diagonal(W).astype(np.float32)[None, :] * K       # [B, D]
    pos = w2 >= 0.0
    perm = np.argsort(~pos, kind="stable")                   # positives first
    hp = int(pos.sum())
    wabs = w1[:, perm] * np.abs(w2[perm])[None, :]           # [D, HID]
    bias12 = (b1[perm] * np.abs(w2[perm])).astype(np.float32)
    has_bias = bool(np.any(bias12 != 0.0))

    w12_all = g[:, :, None] * wabs[None, :, :]               # [B, D, HID]
    wmax = float(np.abs(w12_all).max()) + 1e-30
    s_exp = np.floor(np.log2(224.0 / wmax))
    if has_bias:
        bmax = float(np.abs(bias12).max()) + 1e-30
        s_exp = min(s_exp, np.floor(np.log2(3.0e4 / bmax)))
    S = float(2.0 ** s_exp)

    # ---- token compaction: keep only unmasked tokens, pad to mult of 128
    cnt = (~mask).sum(1)
    HP = max(128, int(np.ceil(cnt.max() / 128.0)) * 128)
    htp = HP // 128
    Vc = np.zeros((B, HP, D), dtype=np.float32)
    mbias = np.full((B, HP), np.float32(MASK_FILL * S), dtype=np.float32)
    for b in range(B):
        idx = np.nonzero(~mask[b])[0]
        Vc[b, :len(idx)] = V[b, idx]
        mbias[b, :len(idx)] = 0.0

    # fp8 gated+scaled weights: [B, 128, DC*HID], chunk c = d rows
    # [c*128, (c+1)*128)
    w12q = np.clip(w12_all * S, -240, 240).astype(E4)
    w12q = np.ascontiguousarray(
        w12q.reshape(B, DC, 128, HID).transpose(0, 2, 1, 3)
    ).reshape(B, 128, DC * HID)

    # fp8 V^T (compacted): [B, 128, DC*HP]
    vt8 = np.clip(Vc, -240, 240).astype(E4).transpose(0, 2, 1)  # [B, D, HP]
    vt8 = np.ascontiguousarray(
        vt8.reshape(B, DC, 128, HP).transpose(0, 2, 1, 3)
    ).reshape(B, 128, DC * HP)

    # fp16 natural V (compacted): [B, 128, htp*D]
    v16 = np.ascontiguousarray(
        Vc.astype(np.float16).reshape(B, htp, 128, D).transpose(0, 2, 1, 3)
    ).reshape(B, 128, htp * D)

    # additive mask bias (pre-scaled by S): [B, 128, htp]
    mbias = np.ascontiguousarray(
        mbias.reshape(B, htp, 128).transpose(0, 2, 1))

    bias_sc = (bias12 * S).astype(np.float16)
    return (vt8, v16, w12q, mbias, bias_sc, has_bias, hp, 1.0 / S,
            float(b2[0]) if b2.size else 0.0, htp)


def _core_maps(vt8, v16, w12q, mbias, bias_sc, has_bias, htp):
    HP = htp * 128
    in_maps = []
    for c in range(NCORES):
        sl = slice(c * BPC, (c + 1) * BPC)
        m = {
            "VT8": np.ascontiguousarray(
                vt8[sl].transpose(1, 0, 2)).reshape(128, BPC * DC * HP),
            "V16": np.ascontiguousarray(
                v16[sl].transpose(1, 0, 2)).reshape(128, BPC * htp * D),
            "W12": np.ascontiguousarray(
                w12q[sl].transpose(1, 0, 2)).reshape(128, BPC * DC * HID),
            "MB": np.ascontiguousarray(
                mbias[sl].transpose(1, 0, 2)).reshape(128, BPC * htp),
        }
        if has_bias:
            m["BI"] = bias_sc.reshape(1, HID)
        in_maps.append(m)
    return in_maps


def kernel(K, V, mask, W, w1, b1, w2, b2):
    from concourse import bass_utils

    vt8, v16, w12q, mbias, bias_sc, has_bias, hp, inv_s, b2val, htp = _prep(
        K, V, mask, W, w1, b1, w2, b2
    )
    nc = _build(hp, b2val, inv_s, has_bias, htp)
    in_maps = _core_maps(vt8, v16, w12q, mbias, bias_sc, has_bias, htp)
    res = bass_utils.run_bass_kernel_spmd(nc, in_maps, core_ids=list(range(NCORES)))
    out = np.concatenate([res.results[c]["OUT"] for c in range(NCORES)], axis=0)
    return out.astype(np.float32)


# revision 5
# speedup vs baseline: 1.1449x; 1.1449x over previous
"""TRN2 Bass kernel for nn_Attention_15590731285136.

Computation (per batch b):
    g      = diag(W) * K[b]                       # [d]
    score  = relu(V[b] @ (g[:,None]*w1) + b1) @ w2 + b2   # [h]
    score  = where(mask[b], MASK_FILL, score)
    alpha  = softmax(score)                        # over h
    out[b] = alpha @ V[b]                          # [d]

Sharding: data-parallel over batch, 8 batches per core on 8 NeuronCores.

Key transformations (v3):
  * Token compaction: masked tokens have alpha == 0 exactly (their score
    is -2^32), so the host gathers only the unmasked tokens of each batch
    (~1024 of 2048) and pads to a multiple of 128. Padding tokens get the
    mask-fill score bias, so their alpha is exactly 0 too. This halves
    the fc1 GEMM, the relu/rowsum work, the alpha@V pass and the DMA
    traffic, with bit-identical math for the surviving tokens.
  * The elementwise gate and w2's magnitudes fold into the weight matrix
    host-side: w12[b] = g[b] * (w1[:, perm] * |w2[perm]|), with a
    sign-grouping permutation (positive-w2 columns first).
  * The fc1 GEMM runs in fp8 (e4m3) with MatmulPerfMode.DoubleRow: each
    matmul contracts TWO 128-deep k-slices per pass, 2x the fp16 rate.
    w12 is scaled by S (power of two) to sit in e4m3's dynamic range;
    softmax is invariant up to the final exp(score/S) which folds 1/S
    into the activation's scale operand.
  * All device-side layouts (compacted V^T fp8, compacted natural V fp16,
    gated w12, additive mask bias) are precomputed host-side, so every
    DMA is a plain contiguous row load.
  * relu+rowsum of fc1 runs fused on ScalarE (ACT, positive-w2 group)
    and VectorE (DVE, negative group) via accum_out; the first KSW token
    tiles' positive group also goes to DVE to balance the two engines.
  * The softmax denominator + alpha@V of batch i are emitted after the
    fc1 loop of batch i+1 (software pipelining) so the PE never waits
    for alpha.
"""

import numpy as np

B, H, D, HID = 64, 2048, 512, 512
NCORES = 8
BPC = B // NCORES          # batches per core
DC = D // 128              # 4 contraction chunks
MASK_FILL = -2.0**32 + 1.0
PRE = 4                    # batches of loads in flight ahead of compute
DEFER = 2                  # batches the softmax tail trails the fc1 loop
KSW = 2                    # leading token tiles whose pos-group runs on DVE


def _build(hp, b2val, inv_s, has_bias, htp):
    import concourse.mybir as mybir
    from concourse import bacc
    from concourse.tile import TileContext

    F32 = mybir.dt.float32
    F16 = mybir.dt.float16
    F8 = mybir.dt.float8e4
    ACTF = mybir.ActivationFunctionType
    ALU = mybir.AluOpType
    DR = mybir.MatmulPerfMode.DoubleRow

    HP = htp * 128             # padded token count
    DCH = DC * HP
    HTD = htp * D
    DCN = DC * HID

    nc = bacc.Bacc(trn_type="TRN2", num_devices=NCORES)

    # all inputs pre-arranged host-side into [128, cols] partition-major
    VT8 = nc.dram_tensor("VT8", (128, BPC * DCH), F8, kind="ExternalInput")
    V16 = nc.dram_tensor("V16", (128, BPC * HTD), F16, kind="ExternalInput")
    W12 = nc.dram_tensor("W12", (128, BPC * DCN), F8, kind="ExternalInput")
    MB = nc.dram_tensor("MB", (128, BPC * htp), F32, kind="ExternalInput")
    if has_bias:
        BI = nc.dram_tensor("BI", (1, HID), F16, kind="ExternalInput")
    OUT = nc.dram_tensor("OUT", (BPC, D), F32, kind="ExternalOutput")

    with TileContext(nc) as tc:
        with (
            tc.tile_pool(name="const", bufs=1) as cpool,
            tc.tile_pool(name="vt", bufs=PRE + 1) as vtpool,
            tc.tile_pool(name="v", bufs=PRE + DEFER + 1) as vpool,
            tc.tile_pool(name="w12", bufs=PRE + 1) as wpool,
            tc.tile_pool(name="scr", bufs=4) as scrpool,
            tc.tile_pool(name="small", bufs=8) as spool,
            tc.tile_pool(name="alpha", bufs=DEFER + 2) as alpool,
            tc.tile_pool(name="fin", bufs=4) as finpool,
            tc.tile_pool(name="fc1_ps", bufs=4, space="PSUM") as fc1ps,
            tc.tile_pool(name="tot_ps", bufs=2, space="PSUM") as totps,
            tc.tile_pool(name="acc_ps", bufs=2, space="PSUM") as accps,
        ):
            def emit_w(bi):
                w12 = wpool.tile([128, DCN], F8, tag="w12")
                nc.sync.dma_start(
                    out=w12, in_=W12.ap()[:, bi * DCN:(bi + 1) * DCN])
                vt = vtpool.tile([128, DCH], F8, tag="vt")
                nc.sync.dma_start(
                    out=vt, in_=VT8.ap()[:, bi * DCH:(bi + 1) * DCH])
                return vt, w12

            def emit_v(bi):
                v = vpool.tile([128, HTD], F16, tag="v")
                nc.gpsimd.dma_start(
                    out=v, in_=V16.ap()[:, bi * HTD:(bi + 1) * HTD])
                return v

            # ---- one-time constants (mall FIRST: tiny + needed by batch 0,
            # must not queue behind the bulk prefetch) ----
            ones_col = cpool.tile([128, 1], F16, tag="ones")
            nc.vector.memset(ones_col, 1.0)
            mall = cpool.tile([128, BPC * htp], F32, tag="mall")
            nc.sync.dma_start(out=mall, in_=MB.ap())
            oball = cpool.tile([1, BPC * D], F32, tag="oball")
            if has_bias:
                ones_row = cpool.tile([1, 128], F16, tag="orr")
                nc.vector.memset(ones_row, 1.0)
                bias_sb = cpool.tile([1, HID], F16, tag="bias")
                nc.sync.dma_start(out=bias_sb, in_=BI.ap())

            pend_w = []
            pend_v = []
            for bi in range(min(PRE, BPC)):
                pend_w.append(emit_w(bi))
                pend_v.append(emit_v(bi))

            def emit_tail(st):
                bi, alpha, v = st
                # alpha @ V
                acc = accps.tile([1, D], F32, tag="acc")
                for j in range(htp):
                    nc.tensor.matmul(
                        out=acc,
                        lhsT=alpha[:, j:j + 1],
                        rhs=v[:, j * D:(j + 1) * D],
                        start=(j == 0),
                        stop=(j == htp - 1),
                    )
                # denominator: sum over all tokens via PE + reduce
                tot = totps.tile([1, htp], F32, tag="tot")
                nc.tensor.matmul(out=tot, lhsT=ones_col, rhs=alpha,
                                 start=True, stop=True)
                tot_sb = finpool.tile([1, 1], F32, tag="tot_sb")
                nc.vector.tensor_reduce(
                    tot_sb, tot, axis=mybir.AxisListType.X, op=ALU.add)
                inv = finpool.tile([1, 1], F32, tag="inv")
                nc.vector.reciprocal(inv, tot_sb)
                nc.vector.tensor_scalar_mul(
                    oball[:, bi * D:(bi + 1) * D], acc, inv)

            deferred = []
            for bi in range(BPC):
                if bi + PRE < BPC:
                    pend_w.append(emit_w(bi + PRE))
                    pend_v.append(emit_v(bi + PRE))
                vt, w12 = pend_w.pop(0)
                v = pend_v.pop(0)
                vt3 = vt.rearrange("p (c h) -> p c h", c=DC)
                w3 = w12.rearrange("p (c n) -> p c n", c=DC)
                mb = mall[:, bi * htp:(bi + 1) * htp]

                sp = spool.tile([128, htp], F32, tag="sp")
                sn = spool.tile([128, htp], F32, tag="sn")
                if hp == 0:
                    nc.vector.memset(sp, 0.0)
                if hp == HID:
                    nc.vector.memset(sn, 0.0)

                # ---- fc1 (fp8 DoubleRow) + fused relu/rowsum per tile ----
                for j in range(htp):
                    fc1 = fc1ps.tile([128, HID], F32, tag="fc1")
                    for pr in range(2):
                        nc.tensor.matmul(
                            out=fc1,
                            lhsT=vt3[:, 2 * pr:2 * pr + 2,
                                     j * 128:(j + 1) * 128],
                            rhs=w3[:, 2 * pr:2 * pr + 2, :],
                            start=(pr == 0),
                            stop=(pr == 1) and not has_bias,
                            perf_mode=DR,
                        )
                    if has_bias:
                        nc.tensor.matmul(
                            out=fc1, lhsT=ones_row, rhs=bias_sb,
                            start=False, stop=True,
                        )
                    if hp > 0:
                        if j < KSW:
                            scrp = scrpool.tile([128, HID], F16, tag="scrp")
                            nc.vector.tensor_scalar(
                                out=scrp[:, :hp], in0=fc1[:, :hp],
                                scalar1=0.0, scalar2=None,
                                op0=ALU.max, op1=ALU.add,
                                accum_out=sp[:, j:j + 1],
                            )
                        else:
                            scra = scrpool.tile([128, HID], F16, tag="scra")
                            nc.scalar.activation(
                                out=scra[:, :hp], in_=fc1[:, :hp],
                                func=ACTF.Relu,
                                accum_out=sp[:, j:j + 1],
                            )
                    if hp < HID:
                        scrd = scrpool.tile([128, HID], F16, tag="scrd")
                        nc.vector.tensor_scalar(
                            out=scrd[:, hp:], in0=fc1[:, hp:],
                            scalar1=0.0, scalar2=None,
                            op0=ALU.max, op1=ALU.add,
                            accum_out=sn[:, j:j + 1],
                        )

                # ---- scores -> masked -> exp(score/S) ----
                sc = spool.tile([128, htp], F32, tag="sc")
                nc.vector.tensor_sub(sc, sp, sn)
                scm = spool.tile([128, htp], F32, tag="scm")
                nc.vector.tensor_add(scm, sc, mb)
                alpha = alpool.tile([128, htp], F16, tag="alpha")
                nc.scalar.activation(
                    out=alpha, in_=scm, func=ACTF.Exp,
                    bias=float(b2val), scale=float(inv_s),
                )

                deferred.append((bi, alpha, v))
                if len(deferred) > DEFER:
                    emit_tail(deferred.pop(0))

            while deferred:
                emit_tail(deferred.pop(0))
            nc.sync.dma_start(
                out=OUT.ap().rearrange("b d -> (b d)")
                    .rearrange("(o f) -> o f", o=1),
                in_=oball)

    nc.finalize()
    return nc


def _prep(K, V, mask, W, w1, b1, w2, b2):
    """Host-side input marshalling (no device work)."""
    import ml_dtypes

    E4 = ml_dtypes.float8_e4m3   # TRN-style e4m3, max normal 240

    K = np.asarray(K, dtype=np.float32)
    V = np.asarray(V, dtype=np.float32)
    mask = np.asarray(mask).astype(bool)
    W = np.asarray(W, dtype=np.float32)
    w1 = np.asarray(w1, dtype=np.float32)
    b1 = np.asarray(b1, dtype=np.float32)
    w2 = np.asarray(w2, dtype=np.float32).reshape(-1)
    b2 = np.asarray(b2, dtype=np.float32).reshape(-1)

    g = np.diagonal(W).astype(np.float32)[None, :] * K       # [B, D]
    pos = w2 >= 0.0
    perm = np.argsort(~pos, kind="stable")                   # positives first
    hp = int(pos.sum())
    wabs = w1[:, perm] * np.abs(w2[perm])[None, :]           # [D, HID]
    bias12 = (b1[perm] * np.abs(w2[perm])).astype(np.float32)
    has_bias = bool(np.any(bias12 != 0.0))

    w12_all = g[:, :, None] * wabs[None, :, :]               # [B, D, HID]
    wmax = float(np.abs(w12_all).max()) + 1e-30
    s_exp = np.floor(np.log2(224.0 / wmax))
    if has_bias:
        bmax = float(np.abs(bias12).max()) + 1e-30
        s_exp = min(s_exp, np.floor(np.log2(3.0e4 / bmax)))
    S = float(2.0 ** s_exp)

    # ---- token compaction: keep only unmasked tokens, pad to mult of 128
    cnt = (~mask).sum(1)
    HP = max(128, int(np.ceil(cnt.max() / 128.0)) * 128)
    htp = HP // 128
    Vc = np.zeros((B, HP, D), dtype=np.float32)
    mbias = np.full((B, HP), np.float32(MASK_FILL * S), dtype=np.float32)
    for b in range(B):
        idx = np.nonzero(~mask[b])[0]
        Vc[b, :len(idx)] = V[b, idx]
        mbias[b, :len(idx)] = 0.0

    # fp8 gated+scaled weights: [B, 128, DC*HID], chunk c = d rows
    # [c*128, (c+1)*128)
    w12q = np.clip(w12_all * S, -240, 240).astype(E4)
    w12q = np.ascontiguousarray(
        w12q.reshape(B, DC, 128, HID).transpose(0, 2, 1, 3)
    ).reshape(B, 128, DC * HID)

    # fp8 V^T (compacted): [B, 128, DC*HP]
    vt8 = np.clip(Vc, -240, 240).astype(E4).transpose(0, 2, 1)  # [B, D, HP]
    vt8 = np.ascontiguousarray(
        vt8.reshape(B, DC, 128, HP).transpose(0, 2, 1, 3)
    ).reshape(B, 128, DC * HP)

    # fp16 natural V (compacted): [B, 128, htp*D]
    v16 = np.ascontiguousarray(
        Vc.astype(np.float16).reshape(B, htp, 128, D).transpose(0, 2, 1, 3)
    ).reshape(B, 128, htp * D)

    # additive mask bias (pre-scaled by S): [B, 128, htp]
    mbias = np.ascontiguousarray(
        mbias.reshape(B, htp, 128).transpose(0, 2, 1))

    bias_sc = (bias12 * S).astype(np.float16)
    return (vt8, v16, w12q, mbias, bias_sc, has_bias, hp, 1.0 / S,
            float(b2[0]) if b2.size else 0.0, htp)


def _core_maps(vt8, v16, w12q, mbias, bias_sc, has_bias, htp):
    HP = htp * 128
    in_maps = []
    for c in range(NCORES):
        sl = slice(c * BPC, (c + 1) * BPC)
        m = {
            "VT8": np.ascontiguousarray(
                vt8[sl].transpose(1, 0, 2)).reshape(128, BPC * DC * HP),
            "V16": np.ascontiguousarray(
                v16[sl].transpose(1, 0, 2)).reshape(128, BPC * htp * D),
            "W12": np.ascontiguousarray(
                w12q[sl].transpose(1, 0, 2)).reshape(128, BPC * DC * HID),
            "MB": np.ascontiguousarray(
                mbias[sl].transpose(1, 0, 2)).reshape(128, BPC * htp),
        }
        if has_bias:
            m["BI"] = bias_sc.reshape(1, HID)
        in_maps.append(m)
    return in_maps


def kernel(K, V, mask, W, w1, b1, w2, b2):
    from concourse import bass_utils

    vt8, v16, w12q, mbias, bias_sc, has_bias, hp, inv_s, b2val, htp = _prep(
        K, V, mask, W, w1, b1, w2, b2
    )
    nc = _build(hp, b2val, inv_s, has_bias, htp)
    in_maps = _core_maps(vt8, v16, w12q, mbias, bias_sc, has_bias, htp)
    res = bass_utils.run_bass_kernel_spmd(nc, in_maps, core_ids=list(range(NCORES)))
    out = np.concatenate([res.results[c]["OUT"] for c in range(NCORES)], axis=0)
    return out.astype(np.float32)


# revision 11
# speedup vs baseline: 1.5334x; 1.3394x over previous
"""TRN2 Bass kernel for nn_Attention_15590731285136.

Computation (per batch b):
    g      = diag(W) * K[b]                       # [d]
    score  = relu(V[b] @ (g[:,None]*w1) + b1) @ w2 + b2   # [h]
    score  = where(mask[b], MASK_FILL, score)
    alpha  = softmax(score)                        # over h
    out[b] = alpha @ V[b]                          # [d]

Sharding: data-parallel over batch, 8 batches per core on 8 NeuronCores.

Key transformations (v3):
  * Token compaction: masked tokens have alpha == 0 exactly (their score
    is -2^32), so the host gathers only the unmasked tokens of each batch
    (~1024 of 2048) and pads to a multiple of 128. Padding tokens get the
    mask-fill score bias, so their alpha is exactly 0 too. This halves
    the fc1 GEMM, the relu/rowsum work, the alpha@V pass and the DMA
    traffic, with bit-identical math for the surviving tokens.
  * The elementwise gate and w2's magnitudes fold into the weight matrix
    host-side: w12[b] = g[b] * (w1[:, perm] * |w2[perm]|), with a
    sign-grouping permutation (positive-w2 columns first).
  * The fc1 GEMM runs in fp8 (e4m3) with MatmulPerfMode.DoubleRow: each
    matmul contracts TWO 128-deep k-slices per pass, 2x the fp16 rate.
    w12 is scaled by S (power of two) to sit in e4m3's dynamic range;
    softmax is invariant up to the final exp(score/S) which folds 1/S
    into the activation's scale operand.
  * All device-side layouts (compacted V^T fp8, compacted natural V fp16,
    gated w12, additive mask bias) are precomputed host-side, so every
    DMA is a plain contiguous row load.
  * relu+rowsum of fc1 runs fused on ScalarE (ACT, positive-w2 group)
    and VectorE (DVE, negative group) via accum_out; the first KSW token
    tiles' positive group also goes to DVE to balance the two engines.
  * The softmax denominator + alpha@V of batch i are emitted after the
    fc1 loop of batch i+1 (software pipelining) so the PE never waits
    for alpha.
"""

import numpy as np

B, H, D, HID = 64, 2048, 512, 512
NCORES = 8
BPC = B // NCORES          # batches per core
DC = D // 128              # 4 contraction chunks
MASK_FILL = -2.0**32 + 1.0
PRE = 2                    # batches of loads emitted ahead of compute
DEFER = 2                  # batches the softmax tail trails the fc1 loop
KSW = 2                    # leading token tiles whose pos-group runs on DVE


def _build(hp, b2val, inv_s, has_bias, htp):
    import concourse.mybir as mybir
    from concourse import bacc
    from concourse.tile import TileContext

    F32 = mybir.dt.float32
    F16 = mybir.dt.float16
    F8 = mybir.dt.float8e4
    ACTF = mybir.ActivationFunctionType
    ALU = mybir.AluOpType
    DR = mybir.MatmulPerfMode.DoubleRow

    HP = htp * 128             # padded token count
    DCH = DC * HP
    HTD = htp * D
    DCN = DC * HID

    nc = bacc.Bacc(trn_type="TRN2", num_devices=NCORES)

    # all inputs pre-arranged host-side into [128, cols] partition-major
    VT8 = nc.dram_tensor("VT8", (128, BPC * DCH), F8, kind="ExternalInput")
    V16 = nc.dram_tensor("V16", (128, BPC * HTD), F16, kind="ExternalInput")
    W12 = nc.dram_tensor("W12", (128, BPC * DCN), F8, kind="ExternalInput")
    MB = nc.dram_tensor("MB", (128, BPC * htp), F32, kind="ExternalInput")
    if has_bias:
        BI = nc.dram_tensor("BI", (1, HID), F16, kind="ExternalInput")
    OUT = nc.dram_tensor("OUT", (BPC, D), F32, kind="ExternalOutput")

    with TileContext(nc) as tc:
        with (
            tc.tile_pool(name="const", bufs=1) as cpool,
            tc.tile_pool(name="vt", bufs=PRE + 3) as vtpool,
            tc.tile_pool(name="v", bufs=PRE + DEFER + 3) as vpool,
            tc.tile_pool(name="w12", bufs=PRE + 2) as wpool,
            tc.tile_pool(name="scr", bufs=4) as scrpool,
            tc.tile_pool(name="small", bufs=8) as spool,
            tc.tile_pool(name="alpha", bufs=DEFER + 2) as alpool,
            tc.tile_pool(name="fin", bufs=4) as finpool,
            tc.tile_pool(name="fc1_ps", bufs=4, space="PSUM") as fc1ps,
            tc.tile_pool(name="tot_ps", bufs=2, space="PSUM") as totps,
            tc.tile_pool(name="acc_ps", bufs=2, space="PSUM") as accps,
        ):
            def emit_w(bi):
                w12 = wpool.tile([128, DCN], F8, tag="w12")
                nc.sync.dma_start(
                    out=w12, in_=W12.ap()[:, bi * DCN:(bi + 1) * DCN])
                vt = vtpool.tile([128, DCH], F8, tag="vt")
                nc.sync.dma_start(
                    out=vt, in_=VT8.ap()[:, bi * DCH:(bi + 1) * DCH])
                return vt, w12

            def emit_v(bi):
                # same logical queue as the w loads: one stream -> FIFO
                # completion order, so v16 traffic can never delay a later
                # batch's fc1-critical w12/vt8 completions via round-robin
                v = vpool.tile([128, HTD], F16, tag="v")
                nc.sync.dma_start(
                    out=v, in_=V16.ap()[:, bi * HTD:(bi + 1) * HTD])
                return v

            # ---- one-time constants (mall FIRST: tiny + needed by batch 0,
            # must not queue behind the bulk prefetch) ----
            ones_col = cpool.tile([128, 1], F16, tag="ones")
            nc.vector.memset(ones_col, 1.0)
            mall = cpool.tile([128, BPC * htp], F32, tag="mall")
            nc.sync.dma_start(out=mall, in_=MB.ap())
            oball = cpool.tile([1, BPC * D], F32, tag="oball")
            if has_bias:
                ones_row = cpool.tile([1, 128], F16, tag="orr")
                nc.vector.memset(ones_row, 1.0)
                bias_sb = cpool.tile([1, HID], F16, tag="bias")
                nc.sync.dma_start(out=bias_sb, in_=BI.ap())

            pend_w = []
            pend_v = []
            for bi in range(min(PRE, BPC)):
                pend_w.append(emit_w(bi))
                pend_v.append(emit_v(bi))
            # later batches' loads are emitted inside the loop AFTER each
            # batch's compute, so no compute instruction's (coarsened)
            # DMA-semaphore wait can ever point at a later batch's load

            def emit_tail(st):
                bi, alpha, v = st
                # alpha @ V
                acc = accps.tile([1, D], F32, tag="acc")
                for j in range(htp):
                    nc.tensor.matmul(
                        out=acc,
                        lhsT=alpha[:, j:j + 1],
                        rhs=v[:, j * D:(j + 1) * D],
                        start=(j == 0),
                        stop=(j == htp - 1),
                    )
                # denominator: sum over all tokens via PE + reduce
                tot = totps.tile([1, htp], F32, tag="tot")
                nc.tensor.matmul(out=tot, lhsT=ones_col, rhs=alpha,
                                 start=True, stop=True)
                tot_sb = finpool.tile([1, 1], F32, tag="tot_sb")
                nc.vector.tensor_reduce(
                    tot_sb, tot, axis=mybir.AxisListType.X, op=ALU.add)
                inv = finpool.tile([1, 1], F32, tag="inv")
                nc.vector.reciprocal(inv, tot_sb)
                nc.vector.tensor_scalar_mul(
                    oball[:, bi * D:(bi + 1) * D], acc, inv)

            deferred = []
            for bi in range(BPC):
                vt, w12 = pend_w.pop(0)
                v = pend_v.pop(0)
                vt3 = vt.rearrange("p (c h) -> p c h", c=DC)
                w3 = w12.rearrange("p (c n) -> p c n", c=DC)
                mb = mall[:, bi * htp:(bi + 1) * htp]

                sp = spool.tile([128, htp], F32, tag="sp")
                sn = spool.tile([128, htp], F32, tag="sn")
                if hp == 0:
                    nc.vector.memset(sp, 0.0)
                if hp == HID:
                    nc.vector.memset(sn, 0.0)

                # ---- fc1 (fp8 DoubleRow) + fused relu/rowsum per tile ----
                for j in range(htp):
                    fc1 = fc1ps.tile([128, HID], F32, tag="fc1")
                    for pr in range(2):
                        nc.tensor.matmul(
                            out=fc1,
                            lhsT=vt3[:, 2 * pr:2 * pr + 2,
                                     j * 128:(j + 1) * 128],
                            rhs=w3[:, 2 * pr:2 * pr + 2, :],
                            start=(pr == 0),
                            stop=(pr == 1) and not has_bias,
                            perf_mode=DR,
                        )
                    if has_bias:
                        nc.tensor.matmul(
                            out=fc1, lhsT=ones_row, rhs=bias_sb,
                            start=False, stop=True,
                        )
                    if hp > 0:
                        if j < KSW:
                            scrp = scrpool.tile([128, HID], F16, tag="scrp")
                            nc.vector.tensor_scalar(
                                out=scrp[:, :hp], in0=fc1[:, :hp],
                                scalar1=0.0, scalar2=None,
                                op0=ALU.max, op1=ALU.add,
                                accum_out=sp[:, j:j + 1],
                            )
                        else:
                            scra = scrpool.tile([128, HID], F16, tag="scra")
                            nc.scalar.activation(
                                out=scra[:, :hp], in_=fc1[:, :hp],
                                func=ACTF.Relu,
                                accum_out=sp[:, j:j + 1],
                            )
                    if hp < HID:
                        scrd = scrpool.tile([128, HID], F16, tag="scrd")
                        nc.vector.tensor_scalar(
                            out=scrd[:, hp:], in0=fc1[:, hp:],
                            scalar1=0.0, scalar2=None,
                            op0=ALU.max, op1=ALU.add,
                            accum_out=sn[:, j:j + 1],
                        )

                # ---- scores -> masked -> exp(score/S) ----
                sc = spool.tile([128, htp], F32, tag="sc")
                nc.vector.tensor_sub(sc, sp, sn)
                scm = spool.tile([128, htp], F32, tag="scm")
                nc.vector.tensor_add(scm, sc, mb)
                alpha = alpool.tile([128, htp], F16, tag="alpha")
                nc.scalar.activation(
                    out=alpha, in_=scm, func=ACTF.Exp,
                    bias=float(b2val), scale=float(inv_s),
                )

                deferred.append((bi, alpha, v))
                if len(deferred) > DEFER:
                    emit_tail(deferred.pop(0))
                if bi + PRE < BPC:
                    pend_w.append(emit_w(bi + PRE))
                    pend_v.append(emit_v(bi + PRE))

            while deferred:
                emit_tail(deferred.pop(0))
            nc.sync.dma_start(
                out=OUT.ap().rearrange("b d -> (b d)")
                    .rearrange("(o f) -> o f", o=1),
                in_=oball)

    nc.finalize()
    return nc


def _prep(K, V, mask, W, w1, b1, w2, b2):
    """Host-side input marshalling (no device work)."""
    import ml_dtypes

    E4 = ml_dtypes.float8_e4m3   # TRN-style e4m3, max normal 240

    K = np.asarray(K, dtype=np.float32)
    V = np.asarray(V, dtype=np.float32)
    mask = np.asarray(mask).astype(bool)
    W = np.asarray(W, dtype=np.float32)
    w1 = np.asarray(w1, dtype=np.float32)
    b1 = np.asarray(b1, dtype=np.float32)
    w2 = np.asarray(w2, dtype=np.float32).reshape(-1)
    b2 = np.asarray(b2, dtype=np.float32).reshape(-1)

    g = np.diagonal(W).astype(np.float32)[None, :] * K       # [B, D]
    pos = w2 >= 0.0
    perm = np.argsort(~pos, kind="stable")                   # positives first
    hp = int(pos.sum())
    wabs = w1[:, perm] * np.abs(w2[perm])[None, :]           # [D, HID]
    bias12 = (b1[perm] * np.abs(w2[perm])).astype(np.float32)
    has_bias = bool(np.any(bias12 != 0.0))

    w12_all = g[:, :, None] * wabs[None, :, :]               # [B, D, HID]
    wmax = float(np.abs(w12_all).max()) + 1e-30
    s_exp = np.floor(np.log2(224.0 / wmax))
    if has_bias:
        bmax = float(np.abs(bias12).max()) + 1e-30
        s_exp = min(s_exp, np.floor(np.log2(3.0e4 / bmax)))
    S = float(2.0 ** s_exp)

    # ---- token compaction: keep only unmasked tokens, pad to mult of 128
    cnt = (~mask).sum(1)
    HP = max(128, int(np.ceil(cnt.max() / 128.0)) * 128)
    htp = HP // 128
    Vc = np.zeros((B, HP, D), dtype=np.float32)
    pad = np.ones((B, HP), dtype=bool)
    for b in range(B):
        idx = np.nonzero(~mask[b])[0]
        Vc[b, :len(idx)] = V[b, idx]
        pad[b, :len(idx)] = False

    # fp8 gated+scaled weights: [B, 128, DC*HID], chunk c = d rows
    # [c*128, (c+1)*128)
    w12q = np.clip(w12_all * S, -240, 240).astype(E4)
    w12q = np.ascontiguousarray(
        w12q.reshape(B, DC, 128, HID).transpose(0, 2, 1, 3)
    ).reshape(B, 128, DC * HID)

    # fp8 V^T (compacted): [B, 128, DC*HP]
    vt8 = np.clip(Vc, -240, 240).astype(E4).transpose(0, 2, 1)  # [B, D, HP]
    vt8 = np.ascontiguousarray(
        vt8.reshape(B, DC, 128, HP).transpose(0, 2, 1, 3)
    ).reshape(B, 128, DC * HP)

    # fp16 natural V (compacted): [B, 128, htp*D]
    v16 = np.ascontiguousarray(
        Vc.astype(np.float16).reshape(B, htp, 128, D).transpose(0, 2, 1, 3)
    ).reshape(B, 128, htp * D)

    # additive mask bias (pre-scaled by S): [B, 128, htp]
    mbias = np.where(pad, np.float32(MASK_FILL * S), np.float32(0.0))
    mbias = np.ascontiguousarray(
        mbias.astype(np.float32).reshape(B, htp, 128).transpose(0, 2, 1))

    bias_sc = (bias12 * S).astype(np.float16)
    return (vt8, v16, w12q, mbias, bias_sc, has_bias, hp, 1.0 / S,
            float(b2[0]) if b2.size else 0.0, htp)


def _core_maps(vt8, v16, w12q, mbias, bias_sc, has_bias, htp):
    HP = htp * 128
    in_maps = []
    for c in range(NCORES):
        sl = slice(c * BPC, (c + 1) * BPC)
        m = {
            "VT8": np.ascontiguousarray(
                vt8[sl].transpose(1, 0, 2)).reshape(128, BPC * DC * HP),
            "V16": np.ascontiguousarray(
                v16[sl].transpose(1, 0, 2)).reshape(128, BPC * htp * D),
            "W12": np.ascontiguousarray(
                w12q[sl].transpose(1, 0, 2)).reshape(128, BPC * DC * HID),
            "MB": np.ascontiguousarray(
                mbias[sl].transpose(1, 0, 2)).reshape(128, BPC * htp),
        }
        if has_bias:
            m["BI"] = bias_sc.reshape(1, HID)
        in_maps.append(m)
    return in_maps


def kernel(K, V, mask, W, w1, b1, w2, b2):
    from concourse import bass_utils

    vt8, v16, w12q, mbias, bias_sc, has_bias, hp, inv_s, b2val, htp = _prep(
        K, V, mask, W, w1, b1, w2, b2
    )
    nc = _build(hp, b2val, inv_s, has_bias, htp)
    in_maps = _core_maps(vt8, v16, w12q, mbias, bias_sc, has_bias, htp)
    res = bass_utils.run_bass_kernel_spmd(nc, in_maps, core_ids=list(range(NCORES)))
    out = np.concatenate([res.results[c]["OUT"] for c in range(NCORES)], axis=0)
    return out.astype(np.float32)
